# revision 52
# baseline (speedup 1.0000x reference)
"""Causal self-attention (B=4, T=2048, C=1024, NH=16) on 8 trn2 NeuronCores.

Sharding: core = (head_group hg in {0,1}) x (batch b in {0..3}).
Each core computes qkv projection + attention + partial output projection for
its 8 heads of its batch; host sums the two head-group partials per batch and
adds the output bias.

v2 layout (all matmul operands bf16; PSUM accumulates f32):
  - q/k computed transposed (qT = W @ x.T, head_size on partitions); V is
    produced directly in natural [token, feat] layout by swapping the matmul
    operands (lhsT = x tile), so no PE transposes.  V bias is folded into the
    host-side output bias via softmax(S) @ (V + 1 b^T) = softmax(S)@V + 1 b^T
    => y += Wproj @ bv, a host constant.
  - S^T = K @ Q^T per (head, 128-key block, 128-query chunk); 8 causal
    blocks packed in one [128, 1024] PSUM tile so one ScalarE Exp covers
    them.  No max subtraction (scores are O(5)).  Causal mask = one 0/1
    multiply per diagonal block on GpSimd.  128-granularity skips the
    fully-masked (key tile 2j+1, query chunk 0) block entirely.
  - AV runs query-on-partitions: acc[q, 0:65] += pt_slice^T @ [V | 1], so
    M=128 (vs 65 in v1) halves AV PE time, and the softmax denominator
    arrives as acc[:, 64] = a per-partition scalar -> 1/d is a cheap DVE
    reciprocal + tensor_scalar_mul (no PE broadcast matmul).  bf16 operands
    keep the 65-row matmuls at 1 cycle/row (fp32r would be 4x at <256).
  - o [q, feat] -> proj-ready oT [feat, q] via ONE dma_start_transpose per
    128-query chunk ([128,512] -> [128,4,128] block-ordered), free of PE/DVE.
  - output projection contracts the core's 512 o-features; partial [T, C]
    result is summed on the host.

Schedule shape (the Tile scheduler is near-in-order per engine, with pool
slots assigned FIFO per tag, so emission order IS the schedule):
  - per 512-token chunk: x DMAs; then per head pair p: q and k m-tiles for
    p, immediately followed by that pair's attention for both query tiles
    (V projection rides between pair 0's m-tiles and its attention).  This
    feeds the Activation engine (exp is ~76% busy, the co-bottleneck) ~15us
    earlier per chunk than finishing qkv first.
  - output projections are deferred to the end of the program with psum
    tiles alternating over the dedicated "py" bank and the by-then-idle
    qkv "mm" banks; they are the PE fill for the ACT-bound late tiles.
  - a few throwaway warmup matmuls burn the PE p-state ramp (0.65->2.4 GHz
    over ~3us) during the dead time before the first x DMA lands.
  - PSUM banks (16KB): mm 2 + S-big 4 + acc 1 + py 1 = 8.
Cost-model span: 203588 ns/core (baseline 309000; PE busy 198us = 97.5%).
"""

import sys

sys.path.insert(0, "/opt/trn_rl_repo")

import numpy as np

import concourse.bacc as bacc
import concourse.bass as bass
import concourse.mybir as mybir
from concourse.bass_utils import run_bass_kernel_spmd
from concourse.tile import TileContext

B, T, C, NH = 4, 2048, 1024, 16
HS = C // NH          # 64
HGF = 512             # features per head group (8 heads x 64)
QT = 256              # query tile (S stage)
NKT = T // 128        # 16 key tiles
GRP = 4               # S-blocks packed per exp instruction
F32 = mybir.dt.float32
BF16 = mybir.dt.bfloat16
Exp = mybir.ActivationFunctionType.Exp


def build_kernel():
    nc = bacc.Bacc(None, target_bir_lowering=False)
    xT = nc.dram_tensor("xT", (C, T), BF16, kind="ExternalInput")
    wqkvT = nc.dram_tensor("wqkvT", (C, 3 * HGF), BF16, kind="ExternalInput")
    bqk = nc.dram_tensor("bqk", (128, 8), F32, kind="ExternalInput")
    wprojT = nc.dram_tensor("wprojT", (HGF, C), BF16, kind="ExternalInput")
    mask01 = nc.dram_tensor("mask01", (128, 128), BF16, kind="ExternalInput")
    y = nc.dram_tensor("y", (T, C), F32, kind="ExternalOutput")

    with TileContext(nc) as tc:
        with (
            tc.tile_pool(name="outer", bufs=1) as outer,
            tc.tile_pool(name="work", bufs=1) as work,
            tc.tile_pool(name="psum", bufs=1, space="PSUM") as psum,
        ):
            # ---- PE p-state warmup: the tensor engine ramps 0.65->2.4 GHz
            # over its first ~3us of activity; burn the ramp on throwaway
            # matmuls during the dead time before the first x DMA lands so
            # real matmuls run at full clock ----
            warm = outer.tile([128, 512], BF16, name="warm")
            nc.vector.memset(warm, 0.0)
            for wi in range(5):
                wtag, wbufs = ("py", 1) if wi % 2 == 0 else ("mm", 2)
                wps = psum.tile([128, 512], F32, tag=wtag, bufs=wbufs,
                                name=f"warm{wi}")
                nc.tensor.matmul(wps, warm[:, 0:128], warm,
                                 start=True, stop=True)

            # ---- resident tensors; bias/mask DMAs are emitted after the
            # first x-chunk below so no queue delays the first qkv matmul ----
            bias_all = outer.tile([128, 8], F32, name="bias_all")
            mask_b = outer.tile([128, 128], BF16, name="mask_b")
            # qkv weights: 512-col chunks spread over queues so the q-part
            # (cols 0:512) lands first, k-part next, v-part last
            w_t = [outer.tile([128, 3 * HGF], BF16, name=f"w{k}") for k in range(8)]
            for k in range(8):
                nc.scalar.dma_start(w_t[k][:, 0:512], wqkvT[k * 128:(k + 1) * 128, 0:512])
            wp_t = [outer.tile([128, C], BF16, name=f"wp{k}") for k in range(4)]

            k_t = [outer.tile([128, T], BF16, name=f"k{i}") for i in range(4)]
            q_sb = [outer.tile([128, T], BF16, name=f"q{i}") for i in range(4)]
            # v_store[i]: [key-tile 128, 8*65]; per head h cols 65h:65h+64 are
            # V features, col 65h+64 is constant 1 (softmax denominator)
            v_store = [outer.tile([128, 8 * 65], BF16, name=f"v{i}") for i in range(NKT)]
            for i in range(NKT):
                nc.vector.memset(
                    v_store[i].rearrange("p (g c) -> p g c", c=65)[:, :, 64:65], 1.0
                )

            oT_all = []
            for n in range(4):  # 512-token chunks
                # ---- qkv projection for chunk n ----
                x_n = []
                for k in range(8):
                    xt = work.tile([128, 512], BF16, tag=f"x{k}", bufs=2,
                                   name=f"x{n}_{k}")
                    # chunk 0 split across two queues so all 8 tiles beat the
                    # first qkv accumulation chain
                    xeng = nc.sync if (n == 0 and k >= 4) else nc.gpsimd
                    xeng.dma_start(
                        xt, xT[k * 128:(k + 1) * 128, n * 512:(n + 1) * 512]
                    )
                    x_n.append(xt)
                if n == 0:
                    # late-needed loads, queued behind the first x chunk:
                    # v-part weights split over both queues (first v matmul
                    # ~5us), then k-part, bias/mask, wp (first proj ~40us)
                    for k in range(8):
                        veng = nc.gpsimd if k < 4 else nc.sync
                        veng.dma_start(
                            w_t[k][:, 1024:1536],
                            wqkvT[k * 128:(k + 1) * 128, 1024:1536],
                        )
                    nc.gpsimd.dma_start(bias_all, bqk[:, :])
                    nc.gpsimd.dma_start(mask_b, mask01[:, :])
                    for k in range(8):
                        nc.sync.dma_start(
                            w_t[k][:, 512:1024],
                            wqkvT[k * 128:(k + 1) * 128, 512:1024],
                        )
                    for k in range(4):
                        nc.sync.dma_start(wp_t[k], wprojT[k * 128:(k + 1) * 128, :])

                # per head pair p: q then k projections for this chunk, then
                # the pair's attention for BOTH query tiles — this feeds the
                # Activation engine exp work ~15us earlier than finishing the
                # whole qkv chunk first.  V projection (natural [token, feat]
                # layout) rides between pair 0's m-tiles and its attention,
                # in time for pair 0's diagonal AV blocks.
                o_sb = {
                    j: [
                        work.tile([128, HGF], BF16, tag=f"os{s}", bufs=4,
                                  name=f"o{j}_{s}")
                        for s in range(2)
                    ]
                    for j in (2 * n, 2 * n + 1)
                }
                for p in range(4):
                    for m in (p, 4 + p):  # q then k, transposed layout
                        ps = psum.tile([128, 512], F32, tag="mm", bufs=2,
                                       name=f"ps{n}_{m}")
                        for k in range(8):
                            nc.tensor.matmul(
                                ps,
                                w_t[k][:, m * 128:(m + 1) * 128],
                                x_n[k],
                                start=(k == 0),
                                stop=(k == 7),
                            )
                        dst = q_sb[m] if m < 4 else k_t[m - 4]
                        nc.vector.tensor_scalar_add(
                            dst[:, n * 512:(n + 1) * 512], ps,
                            bias_all[:, m:m + 1]
                        )
                    if p == 0:
                        for t4 in range(4):
                            vtag, vbufs = (("acc", 1) if n == 0 and t4 == 2
                                           else ("mm", 2))
                            ps = psum.tile([128, 512], F32, tag=vtag,
                                           bufs=vbufs, name=f"psv{n}_{t4}")
                            for k in range(8):
                                nc.tensor.matmul(
                                    ps,
                                    x_n[k][:, t4 * 128:(t4 + 1) * 128],
                                    w_t[k][:, 1024:1536],
                                    start=(k == 0),
                                    stop=(k == 7),
                                )
                            vt = v_store[4 * n + t4]
                            nc.vector.tensor_copy(
                                vt.rearrange("p (g c) -> p g c", c=65)[:, :, 0:64],
                                ps.rearrange("p (g c) -> p g c", c=64),
                            )
                    for j, h in [(2 * n, 2 * p), (2 * n, 2 * p + 1),
                                 (2 * n + 1, 2 * p), (2 * n + 1, 2 * p + 1)]:
                        pair, off = h // 2, 64 * (h % 2)
                        # both 128-query chunks in one bank-sized tile:
                        # s-chunk s accumulates at cols [256s, 256s+65)
                        acc2 = psum.tile([128, 512], F32, tag="acc", bufs=1,
                                         name=f"acc{j}_{h}")
                        acc = [acc2[:, 256 * s:256 * s + 65] for s in range(2)]
                        # 128-wide causal blocks: query chunk s needs key
                        # tiles i <= 2j+s (i == 2j+s is the masked diagonal)
                        blocks = [(s, i) for s in range(2)
                                  for i in range(2 * j + s + 1)]
                        for g in range((len(blocks) + 7) // 8):
                            grp = blocks[8 * g:8 * g + 8]
                            sg = psum.tile([128, 1024], F32, tag="big",
                                           bufs=2, name=f"sg{j}_{h}_{g}")
                            for bi, (s, i) in enumerate(grp):
                                nc.tensor.matmul(
                                    sg[:, bi * 128:(bi + 1) * 128],
                                    k_t[pair][off:off + 64, i * 128:(i + 1) * 128],
                                    q_sb[pair][off:off + 64,
                                               j * QT + s * 128:
                                               j * QT + s * 128 + 128],
                                    start=True,
                                    stop=True,
                                )
                            pt = work.tile([128, 1024], BF16, tag="pt",
                                           bufs=6, name=f"pt{j}_{h}_{g}")
                            nc.scalar.activation(
                                pt[:, :len(grp) * 128], sg[:, :len(grp) * 128],
                                Exp, scale=0.125
                            )
                            for bi, (s, i) in enumerate(grp):
                                if i == 2 * j + s:  # diagonal triangle
                                    nc.gpsimd.tensor_mul(
                                        pt[:, bi * 128:(bi + 1) * 128],
                                        pt[:, bi * 128:(bi + 1) * 128],
                                        mask_b,
                                    )
                            for bi, (s, i) in enumerate(grp):
                                nc.tensor.matmul(
                                    acc[s],
                                    pt[:, bi * 128:(bi + 1) * 128],
                                    v_store[i][:, 65 * h:65 * h + 65],
                                    start=(i == 0),
                                    stop=(i == 2 * j + s),
                                )
                        for s in range(2):
                            dinv = work.tile([128, 1], F32, tag="dinv", bufs=4,
                                             name=f"di{j}_{h}_{s}")
                            nc.vector.reciprocal(dinv, acc[s][:, 64:65])
                            nc.vector.tensor_scalar_mul(
                                o_sb[j][s][:, 64 * h:64 * h + 64],
                                acc[s][:, 0:64],
                                dinv,
                            )
                        del acc2
                # o -> proj-ready layout now (frees o_sb); proj matmuls are
                # deferred to the end as PE fill for the ACT-bound late tiles
                for j in (2 * n, 2 * n + 1):
                    for s in range(2):
                        oT = work.tile([128, 4, 128], BF16, tag="ot", bufs=16,
                                       name=f"ot{j}_{s}")
                        nc.sync.dma_start_transpose(oT, o_sb[j][s])
                        oT_all.append((j, s, oT))

            # ---- deferred output projections; psy alternates between the
            # dedicated "py" bank and the by-now-idle qkv "mm" banks so three
            # psy tiles can be in flight ----
            for idx, (j, s, oT) in enumerate(oT_all):
                for nn in range(2):
                    tag = "py" if (2 * idx + nn) % 3 == 0 else "mm"
                    psy = psum.tile([128, 512], F32, tag=tag,
                                    bufs=(1 if tag == "py" else 2),
                                    name=f"py{j}_{s}_{nn}")
                    for k4 in range(4):
                        nc.tensor.matmul(
                            psy,
                            oT[:, k4:k4 + 1, :],
                            wp_t[k4][:, nn * 512:(nn + 1) * 512],
                            start=(k4 == 0),
                            stop=(k4 == 3),
                        )
                    ysb = work.tile([128, 512], F32, tag="ysb", bufs=6,
                                    name=f"ys{j}_{s}_{nn}")
                    rows = y[j * QT + s * 128:j * QT + (s + 1) * 128, :]
                    if idx == len(oT_all) - 1:
                        # final drain: halves in parallel on two engines and
                        # two DMA queues to shorten the closing chain
                        nc.scalar.activation(
                            ysb[:, 0:256], psy[:, 0:256],
                            mybir.ActivationFunctionType.Copy,
                        )
                        nc.vector.tensor_copy(ysb[:, 256:512], psy[:, 256:512])
                        nc.sync.dma_start(
                            rows[:, nn * 512:nn * 512 + 256], ysb[:, 0:256]
                        )
                        nc.gpsimd.dma_start(
                            rows[:, nn * 512 + 256:nn * 512 + 512],
                            ysb[:, 256:512],
                        )
                    else:
                        if idx >= 13:
                            # exp work is done by the time the last few proj
                            # groups drain; the idle scalar engine halves the
                            # final PSUM->SBUF drain latency
                            nc.scalar.activation(
                                ysb, psy, mybir.ActivationFunctionType.Copy
                            )
                        else:
                            nc.vector.tensor_copy(ysb, psy)
                        yeng = nc.sync if (2 * idx + nn) % 2 == 0 else nc.gpsimd
                        yeng.dma_start(rows[:, nn * 512:(nn + 1) * 512], ysb)

    nc.finalize()
    return nc


_NC = None


def _get_nc():
    global _NC
    if _NC is None:
        _NC = build_kernel()
    return _NC


def kernel(x, Wqkv, bqkv, Wproj, bproj, _trace=False):
    import ml_dtypes

    bf16 = ml_dtypes.bfloat16
    x = np.asarray(x, dtype=np.float32)
    Wqkv = np.asarray(Wqkv, dtype=np.float32)
    bqkv = np.asarray(bqkv, dtype=np.float32)
    Wproj = np.asarray(Wproj, dtype=np.float32)
    bproj = np.asarray(bproj, dtype=np.float32)

    # [key, query] diagonal triangle: allow key <= query
    mask = np.triu(np.ones((128, 128), dtype=np.float32)).astype(bf16)
    in_maps = []
    for hg in range(2):
        sl = slice(hg * HGF, (hg + 1) * HGF)
        rows = np.concatenate(
            [Wqkv[sl], Wqkv[1024 + hg * HGF:1024 + (hg + 1) * HGF],
             Wqkv[2048 + hg * HGF:2048 + (hg + 1) * HGF]]
        )
        wqkvT_np = np.ascontiguousarray(rows.T).astype(bf16)  # [C, 1536]
        bq = np.ascontiguousarray(
            np.concatenate(
                [bqkv[sl], bqkv[1024 + hg * HGF:1024 + (hg + 1) * HGF]]
            ).reshape(8, 128).T
        )
        wprojT_np = np.ascontiguousarray(Wproj[:, sl].T).astype(bf16)  # [512, C]
        for b in range(B):
            in_maps.append(
                {
                    "xT": np.ascontiguousarray(x[b].T).astype(bf16),
                    "wqkvT": wqkvT_np,
                    "bqk": bq,
                    "wprojT": wprojT_np,
                    "mask01": mask,
                }
            )
    # core order: idx = hg * 4 + b
    res = run_bass_kernel_spmd(_get_nc(), in_maps, core_ids=list(range(8)),
                               trace=_trace)
    # V-bias folds into a constant output row: softmax rows sum to 1, so
    # y += (Wproj @ bv) for the full bv (both head groups combined)
    bias_row = bproj + Wproj @ bqkv[2 * C:3 * C]
    out = np.empty((B, T, C), dtype=np.float32)
    for b in range(B):
        out[b] = res.results[b]["y"] + res.results[4 + b]["y"] + bias_row
    if _trace:
        return out, res
    return out
